# revision 21
# baseline (speedup 1.0000x reference)
"""Trainium2 Bass kernel: grouped (depthwise) time-domain cross-correlation.

Problem: data1, data2 [nb=32, nc=64, nt=8192] f32.
out[b,c,l] = sum_t data2[b,c,t] * data1[b,c, t + l - 257]  (data1 zero-padded),
l in [0, 515).   (== F.conv1d groups=nb*nc, padding=257)

Sharding: data-parallel over nb across 8 NeuronCores (4 nb-rows = 256 pairs
each).

Per-core algorithm (per 4-pair group), v4:
  The lag range is split in two halves, l in [0,258) and [258,515), each a
  shear grid w = (l - 258*h) + m of width W_H = 292 < 512 (one PSUM bank).

  Stage 1 (PE): per pair s and half h, 4 accumulating matmuls (one per
  32-phase class b) at tile_position (0, 32s):
      R[32s+m, w] += sum_q x2[128q + 32b + m] * x1[128q + 32b + m + l - 257]
  stationary x2c[q, 32b+m], moving a5[q, j] = x1[128q + j - 257] with
  j = w + 32b + 258h.  The betas accumulate in PSUM, so no beta-reduce
  matmul stage and no extra PSUM drain.  b-outer issue order round-robins
  the 4 pairs' col-groups so 4 matmuls stream concurrently.

  Stage 2 (DVE + ACT): drain both halves into one fp8 tile r4sb[128, 584].
  Stage 3: one DRAM round trip: write [128,584], skewed 4-dim re-read
  (partition stride SCR_W+1) gives SK[32s+m, 260h + l'] = R_h[32s+m, l'+m]
  (DRAM-side access patterns may step 1 element per partition, on-chip ones
  cannot).
  Stage 4 (PE): block-ones BD4 sums the 32 phases, col-tiled across 4
  consecutive groups: out16[32G+j, l'] = sum_m SK...
  Stage 5: drain (DVE+ACT) + 4 stores per 4-group super-iteration.

  DMA issue cost is ~0.65us fixed per dma_start on its queue engine, so
  DMAs are few (5/group) and spread: sync gets a54+skew, scalar(ACT-HWDGE)
  gets x2c+scratch-write, gpsimd(SWDGE) gets the output stores.
"""

import ml_dtypes
import numpy as np

import concourse.bacc as bacc
import concourse.bass as bass
import concourse.mybir as mybir
import concourse.tile as tile
from concourse.bass_utils import run_bass_kernel_spmd

# ---------------- problem constants (hardcoded per spec) ----------------
NB, NCH, NT = 32, 64, 8192
N_CORES = 8
NB_PER_CORE = NB // N_CORES          # 4
PAIRS = NB_PER_CORE * NCH            # 256 pairs per core
OUT_LEN = 515                        # 2*256 + 3
XPAD = 384
X1LEN = NT + 2 * XPAD                # 8960
GROUPS = PAIRS // 4                  # 64 groups of 4 pairs

L_H = (258, 257)                     # lags per half
W_H = 292                            # shear-grid width per half
SK_W = 260                           # skew read width per half
SCR_W = 640                          # scratch row stride (both halves)

A5OFF = 257
A5LEN = 648                          # j in [0, 648): covers w + 32b + 258h

F32 = mybir.dt.float32
BF16 = mybir.dt.bfloat16
S7_DELAY = 6                         # groups between skew issue and s7 use


def _consts():
    bd4 = np.zeros((128, 4), np.float32)
    for p in range(128):
        bd4[p, p // 32] = 1.0
    return bd4


def _build(nc: bass.Bass):
    d1p = nc.dram_tensor("d1p", [PAIRS, X1LEN], BF16, kind="ExternalInput")
    d2 = nc.dram_tensor("d2", [PAIRS, NT], BF16, kind="ExternalInput")
    out = nc.dram_tensor("out", [PAIRS, OUT_LEN], F32, kind="ExternalOutput")

    bd4_np = _consts()
    bd4_dram = nc.inline_tensor(bd4_np.astype(ml_dtypes.bfloat16), name="bd4")

    with tile.TileContext(nc) as tc:
        with (
            tc.tile_pool(name="consts", bufs=1) as consts,
            tc.tile_pool(name="x2p", bufs=4) as x2p,
            tc.tile_pool(name="apool", bufs=6) as apool,
            tc.tile_pool(name="rps", bufs=2, space="PSUM") as rps_pool,
            tc.tile_pool(name="r4sb", bufs=3) as r4sb_pool,
            tc.tile_pool(name="scr", bufs=S7_DELAY + 2, space="DRAM") as scr_pool,
            tc.tile_pool(name="skp", bufs=S7_DELAY + 3) as sk_pool,
            tc.tile_pool(name="out16", bufs=2, space="PSUM") as out16_pool,
            tc.tile_pool(name="outsb", bufs=2) as outsb_pool,
        ):
            bd4 = consts.tile([128, 4], BF16, tag="bd4")
            nc.sync.dma_start(bd4[:], bd4_dram.ap())

            # ---- 8-pair load batches, prefetched 2 groups ahead.
            # Loads sit at the front of each queue's FIFO so the late
            # (scratch/skew) ops behind them can never starve stage 1. ----
            x2tiles = {}
            a5tiles = {}

            def load8(g8):
                # loads pairs for groups g8 (sync queue) and g8+1 (ACT queue)
                p0 = g8 * 4
                x2c8 = x2p.tile([64, 1024], BF16, tag="x2c8")
                nc.gpsimd.dma_start(
                    x2c8[:],
                    bass.AP(d2, p0 * NT, [[128, 64], [NT, 8], [1, 128]]),
                )
                x2tiles[g8] = x2c8
                for half, eng in ((0, nc.sync), (1, nc.scalar)):
                    a54 = apool.tile([64, 4 * A5LEN], BF16, tag="a54")
                    for s in range(4):
                        eng.dma_start(
                            a54[:, s * A5LEN:(s + 1) * A5LEN],
                            bass.AP(
                                d1p,
                                (p0 + 4 * half + s) * X1LEN + XPAD - A5OFF,
                                [[128, 64], [1, A5LEN]],
                            ),
                        )
                    a5tiles[g8 + half] = a54

            load8(0)
            load8(2)
            out16 = None
            sks = {}
            for it in range(GROUPS + S7_DELAY):
                if it < GROUPS:
                    if it % 2 == 0 and it + 4 < GROUPS:
                        load8(it + 4)
                    grp = it
                    p0 = grp * 4
                    x2c4 = x2tiles[grp - grp % 2][:, 512 * (grp % 2):
                                                  512 * (grp % 2) + 512]
                    a54 = a5tiles.pop(grp)

                    # ---- stage 1: 16 accumulating matmuls per half ----
                    # both halves in one 2-bank PSUM tile:
                    #   h -> cols 512h + [0, W_H)
                    rps2 = rps_pool.tile([128, 1024], F32, tag="rps2")
                    for h in range(2):
                        for b in range(4):
                            for s in range(4):
                                ja = A5LEN * s + 32 * b + 258 * h
                                nc.tensor.matmul(
                                    rps2[32 * s:32 * s + 32,
                                         512 * h:512 * h + W_H],
                                    x2c4[:, 128 * s + 32 * b:
                                         128 * s + 32 * b + 32],
                                    a54[:, ja:ja + W_H],
                                    start=(b == 0),
                                    stop=(b == 3),
                                    tile_position=(0, 32 * s),
                                )
                    # ---- stage 2: single combined drain (DVE) ----
                    r4sb = r4sb_pool.tile([128, 2 * W_H], BF16, tag="r4sb")
                    nc.vector.tensor_copy(r4sb[:, 0:W_H], rps2[:, 0:W_H])
                    nc.scalar.copy(
                        r4sb[:, W_H:2 * W_H], rps2[:, 512:512 + W_H]
                    )

                    # ---- stage 3: DRAM round trip, skewed re-reads ----
                    scr = scr_pool.tile([128, SCR_W], BF16, tag="scr")
                    nc.gpsimd.dma_start(scr[:, 0:2 * W_H], r4sb[:])
                    sk = sk_pool.tile([128, 2 * SK_W], BF16, tag="sk")
                    base = scr[:]
                    nc.sync.dma_start(
                        sk[:, 0:SK_W],
                        bass.AP(
                            base.tensor, base.offset,
                            [[32 * SCR_W, 4], [SCR_W + 1, 32], [1, SK_W]],
                        ),
                    )
                    nc.scalar.dma_start(
                        sk[:, SK_W:2 * SK_W],
                        bass.AP(
                            base.tensor, base.offset + W_H,
                            [[32 * SCR_W, 4], [SCR_W + 1, 32], [1, SK_W]],
                        ),
                    )
                    sks[grp] = sk

                if it >= S7_DELAY:
                    # ---- stage 4 (delayed): phase-sum, col-tiled over 4
                    # groups.  By now the skew read is long done, so these
                    # never head-of-line-block the tensor queue. ----
                    dgrp = it - S7_DELAY
                    G = dgrp % 4
                    sk = sks.pop(dgrp)
                    if G == 0:
                        out16 = out16_pool.tile([128, 1024], F32, tag="out16")
                    nc.tensor.matmul(
                        out16[32 * G:32 * G + 4, 0:L_H[0]],
                        bd4[:],
                        sk[:, 0:L_H[0]],
                        start=True, stop=True,
                        tile_position=(0, 32 * G),
                    )
                    nc.tensor.matmul(
                        out16[32 * G:32 * G + 4, 512:512 + L_H[1]],
                        bd4[:],
                        sk[:, SK_W:SK_W + L_H[1]],
                        start=True, stop=True,
                        tile_position=(0, 32 * G),
                    )

                    # ---- stage 5: drain + store 16 pairs ----
                    if G == 3:
                        outsb = outsb_pool.tile([128, 516], F32, tag="outsb")
                        nc.vector.tensor_copy(
                            outsb[:, 0:L_H[0]], out16[:, 0:L_H[0]]
                        )
                        nc.scalar.copy(
                            outsb[:, L_H[0]:OUT_LEN],
                            out16[:, 512:512 + L_H[1]],
                        )
                        for Gs in range(4):
                            q0 = (dgrp - 3 + Gs) * 4
                            nc.gpsimd.dma_start(
                                out.ap()[q0:q0 + 4, :],
                                outsb[32 * Gs:32 * Gs + 4, 0:OUT_LEN],
                            )

    return nc


_NC_CACHE = {}


def _get_nc():
    if "nc" not in _NC_CACHE:
        nc = bacc.Bacc("TRN2", target_bir_lowering=False, debug=False)
        _build(nc)
        nc.compile()
        _NC_CACHE["nc"] = nc
    return _NC_CACHE["nc"]


def _make_in_maps(data1: np.ndarray, data2: np.ndarray):
    data1 = np.asarray(data1, dtype=np.float32).astype(ml_dtypes.bfloat16)
    data2 = np.asarray(data2, dtype=np.float32).astype(ml_dtypes.bfloat16)
    in_maps = []
    for k in range(N_CORES):
        d1 = data1[k * NB_PER_CORE:(k + 1) * NB_PER_CORE].reshape(PAIRS, NT)
        d2 = data2[k * NB_PER_CORE:(k + 1) * NB_PER_CORE].reshape(PAIRS, NT)
        d1p = np.zeros((PAIRS, X1LEN), ml_dtypes.bfloat16)
        d1p[:, XPAD:XPAD + NT] = d1
        in_maps.append({"d1p": d1p, "d2": np.ascontiguousarray(d2)})
    return in_maps


def run(data1: np.ndarray, data2: np.ndarray, trace: bool = False):
    nc = _get_nc()
    in_maps = _make_in_maps(data1, data2)
    res = run_bass_kernel_spmd(
        nc, in_maps, core_ids=list(range(N_CORES)), trace=trace
    )
    outs = [res.results[k]["out"].reshape(NB_PER_CORE, NCH, OUT_LEN)
            for k in range(N_CORES)]
    full = np.concatenate(outs, axis=0).astype(np.float32)
    return full, res


def kernel(data1: np.ndarray, data2: np.ndarray) -> np.ndarray:
    full, _ = run(data1, data2, trace=False)
    return full


# revision 22
# speedup vs baseline: 1.2899x; 1.2899x over previous
"""Trainium2 Bass kernel: grouped (depthwise) time-domain cross-correlation.

Problem: data1, data2 [nb=32, nc=64, nt=8192] f32.
out[b,c,l] = sum_t data2[b,c,t] * data1[b,c, t + l - 257]  (data1 zero-padded),
l in [0, 515).   (== F.conv1d groups=nb*nc, padding=257)

Sharding: data-parallel over nb across 8 NeuronCores (4 nb-rows = 256 pairs
each).

Per-core algorithm (per 4-pair group), v4:
  The lag range is split in two halves, l in [0,258) and [258,515), each a
  shear grid w = (l - 258*h) + m of width W_H = 292 < 512 (one PSUM bank).

  Stage 1 (PE): per pair s and half h, 4 accumulating matmuls (one per
  32-phase class b) at tile_position (0, 32s):
      R[32s+m, w] += sum_q x2[128q + 32b + m] * x1[128q + 32b + m + l - 257]
  stationary x2c[q, 32b+m], moving a5[q, j] = x1[128q + j - 257] with
  j = w + 32b + 258h.  The betas accumulate in PSUM, so no beta-reduce
  matmul stage and no extra PSUM drain.  b-outer issue order round-robins
  the 4 pairs' col-groups so 4 matmuls stream concurrently.

  Stage 2 (DVE + ACT): drain both halves into one fp8 tile r4sb[128, 584].
  Stage 3: one DRAM round trip: write [128,584], skewed 4-dim re-read
  (partition stride SCR_W+1) gives SK[32s+m, 260h + l'] = R_h[32s+m, l'+m]
  (DRAM-side access patterns may step 1 element per partition, on-chip ones
  cannot).
  Stage 4 (PE): block-ones BD4 sums the 32 phases, col-tiled across 4
  consecutive groups: out16[32G+j, l'] = sum_m SK...
  Stage 5: drain (DVE+ACT) + 4 stores per 4-group super-iteration.

  DMA issue cost is ~0.65us fixed per dma_start on its queue engine, so
  DMAs are few (5/group) and spread: sync gets a54+skew, scalar(ACT-HWDGE)
  gets x2c+scratch-write, gpsimd(SWDGE) gets the output stores.
"""

import ml_dtypes
import numpy as np

import concourse.bacc as bacc
import concourse.bass as bass
import concourse.mybir as mybir
import concourse.tile as tile
from concourse.bass_utils import run_bass_kernel_spmd

# ---------------- problem constants (hardcoded per spec) ----------------
NB, NCH, NT = 32, 64, 8192
N_CORES = 8
NB_PER_CORE = NB // N_CORES          # 4
PAIRS = NB_PER_CORE * NCH            # 256 pairs per core
OUT_LEN = 515                        # 2*256 + 3
XPAD = 384
X1LEN = NT + 2 * XPAD                # 8960
GROUPS = PAIRS // 4                  # 64 groups of 4 pairs

L_H = (258, 257)                     # lags per half
W_H = 292                            # shear-grid width per half
SK_W = 260                           # skew read width per half
SCR_W = 640                          # scratch row stride (both halves)

A5OFF = 257
A5LEN = 648                          # j in [0, 648): covers w + 32b + 258h

F32 = mybir.dt.float32
BF16 = mybir.dt.bfloat16
S7_DELAY = 6                         # groups between skew issue and s7 use


def _consts():
    bd4 = np.zeros((128, 4), np.float32)
    for p in range(128):
        bd4[p, p // 32] = 1.0
    return bd4


def _build(nc: bass.Bass):
    d1p = nc.dram_tensor("d1p", [PAIRS, X1LEN], BF16, kind="ExternalInput")
    d2 = nc.dram_tensor("d2", [PAIRS, NT], BF16, kind="ExternalInput")
    out = nc.dram_tensor("out", [PAIRS, OUT_LEN], F32, kind="ExternalOutput")

    bd4_np = _consts()
    bd4_dram = nc.inline_tensor(bd4_np.astype(ml_dtypes.bfloat16), name="bd4")

    with tile.TileContext(nc) as tc:
        with (
            tc.tile_pool(name="consts", bufs=1) as consts,
            tc.tile_pool(name="x2p", bufs=4) as x2p,
            tc.tile_pool(name="apool", bufs=6) as apool,
            tc.tile_pool(name="rps", bufs=3, space="PSUM") as rps_pool,
            tc.tile_pool(name="r4sb", bufs=3) as r4sb_pool,
            tc.tile_pool(name="scr", bufs=S7_DELAY + 2, space="DRAM") as scr_pool,
            tc.tile_pool(name="skp", bufs=S7_DELAY + 3) as sk_pool,
            tc.tile_pool(name="out16", bufs=1, space="PSUM") as out16_pool,
            tc.tile_pool(name="outsb", bufs=2) as outsb_pool,
        ):
            bd4 = consts.tile([128, 4], BF16, tag="bd4")
            nc.sync.dma_start(bd4[:], bd4_dram.ap())

            # ---- 8-pair load batches, prefetched 2 groups ahead.
            # Loads sit at the front of each queue's FIFO so the late
            # (scratch/skew) ops behind them can never starve stage 1. ----
            x2tiles = {}
            a5tiles = {}

            def load8(g8):
                # loads pairs for groups g8 (sync queue) and g8+1 (ACT queue)
                p0 = g8 * 4
                x2c8 = x2p.tile([64, 1024], BF16, tag="x2c8")
                nc.gpsimd.dma_start(
                    x2c8[:],
                    bass.AP(d2, p0 * NT, [[128, 64], [NT, 8], [1, 128]]),
                )
                x2tiles[g8] = x2c8
                for half, eng in ((0, nc.sync), (1, nc.scalar)):
                    a54 = apool.tile([64, 4 * A5LEN], BF16, tag="a54")
                    for s in range(4):
                        eng.dma_start(
                            a54[:, s * A5LEN:(s + 1) * A5LEN],
                            bass.AP(
                                d1p,
                                (p0 + 4 * half + s) * X1LEN + XPAD - A5OFF,
                                [[128, 64], [1, A5LEN]],
                            ),
                        )
                    a5tiles[g8 + half] = a54

            load8(0)
            load8(2)
            out16 = None
            sks = {}
            for it in range(GROUPS + S7_DELAY):
                if it < GROUPS:
                    if it % 2 == 0 and it + 4 < GROUPS:
                        load8(it + 4)
                    grp = it
                    p0 = grp * 4
                    x2c4 = x2tiles[grp - grp % 2][:, 512 * (grp % 2):
                                                  512 * (grp % 2) + 512]
                    a54 = a5tiles.pop(grp)

                    # ---- stage 1: 16 accumulating matmuls per half ----
                    # both halves in one 2-bank PSUM tile:
                    #   h -> cols 512h + [0, W_H)
                    rps2 = rps_pool.tile([128, 1024], F32, tag="rps2")
                    for h in range(2):
                        for b in range(4):
                            for s in range(4):
                                ja = A5LEN * s + 32 * b + 258 * h
                                nc.tensor.matmul(
                                    rps2[32 * s:32 * s + 32,
                                         512 * h:512 * h + W_H],
                                    x2c4[:, 128 * s + 32 * b:
                                         128 * s + 32 * b + 32],
                                    a54[:, ja:ja + W_H],
                                    start=(b == 0),
                                    stop=(b == 3),
                                    tile_position=(0, 32 * s),
                                )
                    # ---- stage 2: single combined drain (DVE) ----
                    r4sb = r4sb_pool.tile([128, 2 * W_H], BF16, tag="r4sb")
                    nc.vector.tensor_copy(
                        r4sb[:].rearrange("p (h v) -> p h v", h=2),
                        rps2[:].rearrange("p (h v) -> p h v", h=2)
                        [:, :, 0:W_H],
                    )

                    # ---- stage 3: DRAM round trip, skewed re-reads ----
                    scr = scr_pool.tile([128, SCR_W], BF16, tag="scr")
                    nc.gpsimd.dma_start(scr[:, 0:2 * W_H], r4sb[:])
                    sk = sk_pool.tile([128, 2 * SK_W], BF16, tag="sk")
                    base = scr[:]
                    nc.sync.dma_start(
                        sk[:, 0:SK_W],
                        bass.AP(
                            base.tensor, base.offset,
                            [[32 * SCR_W, 4], [SCR_W + 1, 32], [1, SK_W]],
                        ),
                    )
                    nc.scalar.dma_start(
                        sk[:, SK_W:2 * SK_W],
                        bass.AP(
                            base.tensor, base.offset + W_H,
                            [[32 * SCR_W, 4], [SCR_W + 1, 32], [1, SK_W]],
                        ),
                    )
                    sks[grp] = sk

                if it >= S7_DELAY:
                    # ---- stage 4 (delayed): phase-sum, col-tiled over 4
                    # groups.  By now the skew read is long done, so these
                    # never head-of-line-block the tensor queue. ----
                    dgrp = it - S7_DELAY
                    G = dgrp % 4
                    sk = sks.pop(dgrp)
                    if G == 0:
                        out16 = out16_pool.tile([128, 1024], F32, tag="out16")
                    nc.tensor.matmul(
                        out16[32 * G:32 * G + 4, 0:L_H[0]],
                        bd4[:],
                        sk[:, 0:L_H[0]],
                        start=True, stop=True,
                        tile_position=(0, 32 * G),
                    )
                    nc.tensor.matmul(
                        out16[32 * G:32 * G + 4, 512:512 + L_H[1]],
                        bd4[:],
                        sk[:, SK_W:SK_W + L_H[1]],
                        start=True, stop=True,
                        tile_position=(0, 32 * G),
                    )

                    # ---- stage 5: drain + store 16 pairs ----
                    if G == 3:
                        outsb = outsb_pool.tile([128, 516], F32, tag="outsb")
                        nc.vector.tensor_copy(
                            outsb[:].rearrange("p (h v) -> p h v", h=2),
                            out16[:].rearrange("p (h v) -> p h v", h=2)
                            [:, :, 0:L_H[0]],
                        )
                        for Gs in range(4):
                            q0 = (dgrp - 3 + Gs) * 4
                            nc.gpsimd.dma_start(
                                out.ap()[q0:q0 + 4, :],
                                outsb[32 * Gs:32 * Gs + 4, 0:OUT_LEN],
                            )

    return nc


_NC_CACHE = {}


def _get_nc():
    if "nc" not in _NC_CACHE:
        nc = bacc.Bacc("TRN2", target_bir_lowering=False, debug=False)
        _build(nc)
        nc.compile()
        _NC_CACHE["nc"] = nc
    return _NC_CACHE["nc"]


def _make_in_maps(data1: np.ndarray, data2: np.ndarray):
    data1 = np.asarray(data1, dtype=np.float32).astype(ml_dtypes.bfloat16)
    data2 = np.asarray(data2, dtype=np.float32).astype(ml_dtypes.bfloat16)
    in_maps = []
    for k in range(N_CORES):
        d1 = data1[k * NB_PER_CORE:(k + 1) * NB_PER_CORE].reshape(PAIRS, NT)
        d2 = data2[k * NB_PER_CORE:(k + 1) * NB_PER_CORE].reshape(PAIRS, NT)
        d1p = np.zeros((PAIRS, X1LEN), ml_dtypes.bfloat16)
        d1p[:, XPAD:XPAD + NT] = d1
        in_maps.append({"d1p": d1p, "d2": np.ascontiguousarray(d2)})
    return in_maps


def run(data1: np.ndarray, data2: np.ndarray, trace: bool = False):
    nc = _get_nc()
    in_maps = _make_in_maps(data1, data2)
    res = run_bass_kernel_spmd(
        nc, in_maps, core_ids=list(range(N_CORES)), trace=trace
    )
    outs = [res.results[k]["out"].reshape(NB_PER_CORE, NCH, OUT_LEN)
            for k in range(N_CORES)]
    full = np.concatenate(outs, axis=0).astype(np.float32)
    return full, res


def kernel(data1: np.ndarray, data2: np.ndarray) -> np.ndarray:
    full, _ = run(data1, data2, trace=False)
    return full


# revision 25
# speedup vs baseline: 1.3638x; 1.0573x over previous
"""Trainium2 Bass kernel: grouped (depthwise) time-domain cross-correlation.

Problem: data1, data2 [nb=32, nc=64, nt=8192] f32.
out[b,c,l] = sum_t data2[b,c,t] * data1[b,c, t + l - 257]  (data1 zero-padded),
l in [0, 515).   (== F.conv1d groups=nb*nc, padding=257)

Sharding: data-parallel over nb across 8 NeuronCores (4 nb-rows = 256 pairs
each).

Per-core algorithm (per 4-pair group), final:
  The lag range is split in two halves, l in [0,258) and [258,515), each a
  shear grid w = (l - 258*h) + m of width W_H = 292 < 512 (one PSUM bank).

  Stage 1 (PE): per pair s and half h, 4 accumulating matmuls (one per
  32-phase class b) at tile_position (0, 32s):
      R[32s+m, w] += sum_q x2[128q + 32b + m] * x1[128q + 32b + m + l - 257]
  stationary x2c[q, 32b+m], moving a5[q, j] = x1[128q + j - 257] with
  j = w + 32b + 258h.  The betas accumulate in PSUM (same cells), so there
  is no beta-reduce matmul stage and no extra PSUM drain.  b-outer issue
  order round-robins the 4 pairs' col-groups so 4 matmuls stream
  concurrently.

  Stage 2 (DVE): one 3-dim copy drains both halves to bf16 r4sb[128, 584].
  Stage 3: DRAM round trip: write [128,584], two skewed re-reads with
  partition stride SCR_W+1 give SK[32s+m, 260h + l'] = R_h[32s+m, l'+m]
  (DRAM-side access patterns may step 1 element per partition, on-chip
  ones cannot).
  Stage 4 (PE), delayed S7_DELAY groups so it never head-of-line-blocks
  the tensor queue: block-ones BD4 sums the 32 phases, col-tiled across 4
  consecutive groups: out16[32G+j, l'] = sum_m SK_j[m, l'].
  Stage 5: drain + 4 stores per 4-group super-iteration.

  Scheduling (the measured constraints that shaped this):
  - each dma_start costs ~0.65us of issue time on its queue engine and
    only three queues exist (sync/scalar HWDGE, gpsimd SWDGE), so DMAs are
    few, batched (8-pair loads), and byte-balanced across the queues;
  - queue FIFOs are in-order, so loads are prefetched 4 groups ahead of
    use and never sit behind drain-dependent scratch/skew ops;
  - stage-4 matmuls run S7_DELAY groups late because the drain->scratch->
    skew chain takes ~2.5 group-times to land in SBUF.
"""

import ml_dtypes
import numpy as np

import concourse.bacc as bacc
import concourse.bass as bass
import concourse.mybir as mybir
import concourse.tile as tile
from concourse.bass_utils import run_bass_kernel_spmd

# ---------------- problem constants (hardcoded per spec) ----------------
NB, NCH, NT = 32, 64, 8192
N_CORES = 8
NB_PER_CORE = NB // N_CORES          # 4
PAIRS = NB_PER_CORE * NCH            # 256 pairs per core
OUT_LEN = 515                        # 2*256 + 3
XPAD = 384
X1LEN = NT + 2 * XPAD                # 8960
GROUPS = PAIRS // 4                  # 64 groups of 4 pairs

L_H = (258, 257)                     # lags per half
W_H = 292                            # shear-grid width per half
SK_W = 260                           # skew read width per half
SCR_W = 640                          # scratch row stride (both halves)

A5OFF = 257
A5LEN = 648                          # j in [0, 648): covers w + 32b + 258h

F32 = mybir.dt.float32
BF16 = mybir.dt.bfloat16
S7_DELAY = 10                        # groups between skew issue and s7 use


def _consts():
    bd4 = np.zeros((128, 4), np.float32)
    for p in range(128):
        bd4[p, p // 32] = 1.0
    return bd4


def _build(nc: bass.Bass):
    d1p = nc.dram_tensor("d1p", [PAIRS, X1LEN], BF16, kind="ExternalInput")
    d2 = nc.dram_tensor("d2", [PAIRS, NT], BF16, kind="ExternalInput")
    out = nc.dram_tensor("out", [PAIRS, OUT_LEN], F32, kind="ExternalOutput")

    bd4_np = _consts()
    bd4_dram = nc.inline_tensor(bd4_np.astype(ml_dtypes.bfloat16), name="bd4")

    with tile.TileContext(nc) as tc:
        with (
            tc.tile_pool(name="consts", bufs=1) as consts,
            tc.tile_pool(name="x2p", bufs=5) as x2p,
            tc.tile_pool(name="apool", bufs=8) as apool,
            tc.tile_pool(name="rps", bufs=3, space="PSUM") as rps_pool,
            tc.tile_pool(name="r4sb", bufs=3) as r4sb_pool,
            tc.tile_pool(name="scr", bufs=S7_DELAY + 2, space="DRAM") as scr_pool,
            tc.tile_pool(name="skp", bufs=S7_DELAY + 3) as sk_pool,
            tc.tile_pool(name="out16", bufs=1, space="PSUM") as out16_pool,
            tc.tile_pool(name="outsb", bufs=2) as outsb_pool,
        ):
            bd4 = consts.tile([128, 4], BF16, tag="bd4")
            nc.sync.dma_start(bd4[:], bd4_dram.ap())

            # ---- 8-pair load batches, prefetched 2 groups ahead.
            # Loads sit at the front of each queue's FIFO so the late
            # (scratch/skew) ops behind them can never starve stage 1. ----
            x2tiles = {}
            a5tiles = {}

            def load8(g8):
                # loads pairs for groups g8 (sync queue) and g8+1 (ACT queue)
                p0 = g8 * 4
                x2c8 = x2p.tile([64, 1024], BF16, tag="x2c8")
                nc.gpsimd.dma_start(
                    x2c8[:],
                    bass.AP(d2, p0 * NT, [[128, 64], [NT, 8], [1, 128]]),
                )
                x2tiles[g8] = x2c8
                for half, eng in ((0, nc.sync), (1, nc.scalar)):
                    a54 = apool.tile([64, 4 * A5LEN], BF16, tag="a54")
                    eng.dma_start(
                        a54[:],
                        bass.AP(
                            d1p,
                            (p0 + 4 * half) * X1LEN + XPAD - A5OFF,
                            [[128, 64], [X1LEN, 4], [1, A5LEN]],
                        ),
                    )
                    a5tiles[g8 + half] = a54

            load8(0)
            load8(2)
            load8(4)
            out16 = None
            sks = {}
            for it in range(GROUPS + S7_DELAY):
                if it < GROUPS:
                    if it % 2 == 0 and it + 6 < GROUPS:
                        load8(it + 6)
                    grp = it
                    p0 = grp * 4
                    x2c4 = x2tiles[grp - grp % 2][:, 512 * (grp % 2):
                                                  512 * (grp % 2) + 512]
                    a54 = a5tiles.pop(grp)

                    # ---- stage 1: 16 accumulating matmuls per half ----
                    # both halves in one 2-bank PSUM tile:
                    #   h -> cols 512h + [0, W_H)
                    rps2 = rps_pool.tile([128, 1024], F32, tag="rps2")
                    for h in range(2):
                        for b in range(4):
                            for s in range(4):
                                ja = A5LEN * s + 32 * b + 258 * h
                                nc.tensor.matmul(
                                    rps2[32 * s:32 * s + 32,
                                         512 * h:512 * h + W_H],
                                    x2c4[:, 128 * s + 32 * b:
                                         128 * s + 32 * b + 32],
                                    a54[:, ja:ja + W_H],
                                    start=(b == 0),
                                    stop=(b == 3),
                                    tile_position=(0, 32 * s),
                                )
                    # ---- stage 2: single combined drain (DVE) ----
                    r4sb = r4sb_pool.tile([128, 2 * W_H], BF16, tag="r4sb")
                    nc.vector.tensor_copy(
                        r4sb[:].rearrange("p (h v) -> p h v", h=2),
                        rps2[:].rearrange("p (h v) -> p h v", h=2)
                        [:, :, 0:W_H],
                    )

                    # ---- stage 3: DRAM round trip, skewed re-reads ----
                    scr = scr_pool.tile([128, SCR_W], BF16, tag="scr")
                    nc.gpsimd.dma_start(scr[:, 0:2 * W_H], r4sb[:])
                    sk = sk_pool.tile([128, 2 * SK_W], BF16, tag="sk")
                    base = scr[:]
                    nc.sync.dma_start(
                        sk[:, 0:SK_W],
                        bass.AP(
                            base.tensor, base.offset,
                            [[32 * SCR_W, 4], [SCR_W + 1, 32], [1, SK_W]],
                        ),
                    )
                    nc.scalar.dma_start(
                        sk[:, SK_W:2 * SK_W],
                        bass.AP(
                            base.tensor, base.offset + W_H,
                            [[32 * SCR_W, 4], [SCR_W + 1, 32], [1, SK_W]],
                        ),
                    )
                    sks[grp] = sk

                if it >= S7_DELAY:
                    # ---- stage 4 (delayed): phase-sum, col-tiled over 4
                    # groups.  By now the skew read is long done, so these
                    # never head-of-line-block the tensor queue. ----
                    dgrp = it - S7_DELAY
                    G = dgrp % 4
                    sk = sks.pop(dgrp)
                    if G == 0:
                        out16 = out16_pool.tile([128, 1024], F32, tag="out16")
                    nc.tensor.matmul(
                        out16[32 * G:32 * G + 4, 0:L_H[0]],
                        bd4[:],
                        sk[:, 0:L_H[0]],
                        start=True, stop=True,
                        tile_position=(0, 32 * G),
                    )
                    nc.tensor.matmul(
                        out16[32 * G:32 * G + 4, 512:512 + L_H[1]],
                        bd4[:],
                        sk[:, SK_W:SK_W + L_H[1]],
                        start=True, stop=True,
                        tile_position=(0, 32 * G),
                    )

                    # ---- stage 5: drain + store 16 pairs ----
                    if G == 3:
                        outsb = outsb_pool.tile([128, 516], F32, tag="outsb")
                        nc.vector.tensor_copy(
                            outsb[:].rearrange("p (h v) -> p h v", h=2),
                            out16[:].rearrange("p (h v) -> p h v", h=2)
                            [:, :, 0:L_H[0]],
                        )
                        for Gs in range(4):
                            q0 = (dgrp - 3 + Gs) * 4
                            nc.gpsimd.dma_start(
                                out.ap()[q0:q0 + 4, :],
                                outsb[32 * Gs:32 * Gs + 4, 0:OUT_LEN],
                            )

    return nc


_NC_CACHE = {}


def _get_nc():
    if "nc" not in _NC_CACHE:
        nc = bacc.Bacc("TRN2", target_bir_lowering=False, debug=False)
        _build(nc)
        nc.compile()
        _NC_CACHE["nc"] = nc
    return _NC_CACHE["nc"]


def _make_in_maps(data1: np.ndarray, data2: np.ndarray):
    data1 = np.asarray(data1, dtype=np.float32).astype(ml_dtypes.bfloat16)
    data2 = np.asarray(data2, dtype=np.float32).astype(ml_dtypes.bfloat16)
    in_maps = []
    for k in range(N_CORES):
        d1 = data1[k * NB_PER_CORE:(k + 1) * NB_PER_CORE].reshape(PAIRS, NT)
        d2 = data2[k * NB_PER_CORE:(k + 1) * NB_PER_CORE].reshape(PAIRS, NT)
        d1p = np.zeros((PAIRS, X1LEN), ml_dtypes.bfloat16)
        d1p[:, XPAD:XPAD + NT] = d1
        in_maps.append({"d1p": d1p, "d2": np.ascontiguousarray(d2)})
    return in_maps


def run(data1: np.ndarray, data2: np.ndarray, trace: bool = False):
    nc = _get_nc()
    in_maps = _make_in_maps(data1, data2)
    res = run_bass_kernel_spmd(
        nc, in_maps, core_ids=list(range(N_CORES)), trace=trace
    )
    outs = [res.results[k]["out"].reshape(NB_PER_CORE, NCH, OUT_LEN)
            for k in range(N_CORES)]
    full = np.concatenate(outs, axis=0).astype(np.float32)
    return full, res


def kernel(data1: np.ndarray, data2: np.ndarray) -> np.ndarray:
    full, _ = run(data1, data2, trace=False)
    return full


# revision 26
# speedup vs baseline: 1.3834x; 1.0144x over previous
"""Trainium2 Bass kernel: grouped (depthwise) time-domain cross-correlation.

Problem: data1, data2 [nb=32, nc=64, nt=8192] f32.
out[b,c,l] = sum_t data2[b,c,t] * data1[b,c, t + l - 257]  (data1 zero-padded),
l in [0, 515).   (== F.conv1d groups=nb*nc, padding=257)

Sharding: data-parallel over nb across 8 NeuronCores (4 nb-rows = 256 pairs
each).

Per-core algorithm (per 4-pair group), final:
  The lag range is split in two halves, l in [0,258) and [258,515), each a
  shear grid w = (l - 258*h) + m of width W_H = 292 < 512 (one PSUM bank).

  Stage 1 (PE): per pair s and half h, 4 accumulating matmuls (one per
  32-phase class b) at tile_position (0, 32s):
      R[32s+m, w] += sum_q x2[128q + 32b + m] * x1[128q + 32b + m + l - 257]
  stationary x2c[q, 32b+m], moving a5[q, j] = x1[128q + j - 257] with
  j = w + 32b + 258h.  The betas accumulate in PSUM (same cells), so there
  is no beta-reduce matmul stage and no extra PSUM drain.  b-outer issue
  order round-robins the 4 pairs' col-groups so 4 matmuls stream
  concurrently.

  Stage 2 (DVE): one 3-dim copy drains both halves to bf16 r4sb[128, 584].
  Stage 3: DRAM round trip: write [128,584], two skewed re-reads with
  partition stride SCR_W+1 give SK[32s+m, 260h + l'] = R_h[32s+m, l'+m]
  (DRAM-side access patterns may step 1 element per partition, on-chip
  ones cannot).
  Stage 4 (PE), delayed S7_DELAY groups so it never head-of-line-blocks
  the tensor queue: block-ones BD4 sums the 32 phases, col-tiled across 4
  consecutive groups: out16[32G+j, l'] = sum_m SK_j[m, l'].
  Stage 5: drain + 4 stores per 4-group super-iteration.

  Scheduling (the measured constraints that shaped this):
  - each dma_start costs ~0.65us of issue time on its queue engine and
    only three queues exist (sync/scalar HWDGE, gpsimd SWDGE), so DMAs are
    few, batched (8-pair loads), and byte-balanced across the queues;
  - queue FIFOs are in-order, so loads are prefetched 4 groups ahead of
    use and never sit behind drain-dependent scratch/skew ops;
  - stage-4 matmuls run S7_DELAY groups late because the drain->scratch->
    skew chain takes ~2.5 group-times to land in SBUF.
"""

import ml_dtypes
import numpy as np

import concourse.bacc as bacc
import concourse.bass as bass
import concourse.mybir as mybir
import concourse.tile as tile
from concourse.bass_utils import run_bass_kernel_spmd

# ---------------- problem constants (hardcoded per spec) ----------------
NB, NCH, NT = 32, 64, 8192
N_CORES = 8
NB_PER_CORE = NB // N_CORES          # 4
PAIRS = NB_PER_CORE * NCH            # 256 pairs per core
OUT_LEN = 515                        # 2*256 + 3
XPAD = 384
X1LEN = NT + 2 * XPAD                # 8960
GROUPS = PAIRS // 4                  # 64 groups of 4 pairs

L_H = (258, 257)                     # lags per half
W_H = 292                            # shear-grid width per half
SK_W = 260                           # skew read width per half
SCR_W = 640                          # scratch row stride (both halves)

A5OFF = 257
A5LEN = 648                          # j in [0, 648): covers w + 32b + 258h

F32 = mybir.dt.float32
BF16 = mybir.dt.bfloat16
S7_DELAY = 10                        # groups between skew issue and s7 use


def _consts():
    bd4 = np.zeros((128, 4), np.float32)
    for p in range(128):
        bd4[p, p // 32] = 1.0
    return bd4


def _build(nc: bass.Bass):
    d1p = nc.dram_tensor("d1p", [PAIRS, X1LEN], BF16, kind="ExternalInput")
    d2 = nc.dram_tensor("d2", [PAIRS, NT], BF16, kind="ExternalInput")
    out = nc.dram_tensor("out", [PAIRS, OUT_LEN], F32, kind="ExternalOutput")

    bd4_np = _consts()
    bd4_dram = nc.inline_tensor(bd4_np.astype(ml_dtypes.bfloat16), name="bd4")

    with tile.TileContext(nc) as tc:
        with (
            tc.tile_pool(name="consts", bufs=1) as consts,
            tc.tile_pool(name="x2p", bufs=5) as x2p,
            tc.tile_pool(name="apool", bufs=8) as apool,
            tc.tile_pool(name="rps", bufs=3, space="PSUM") as rps_pool,
            tc.tile_pool(name="r4sb", bufs=3) as r4sb_pool,
            tc.tile_pool(name="scr", bufs=S7_DELAY + 2, space="DRAM") as scr_pool,
            tc.tile_pool(name="skp", bufs=S7_DELAY + 3) as sk_pool,
            tc.tile_pool(name="out16", bufs=1, space="PSUM") as out16_pool,
            tc.tile_pool(name="outsb", bufs=2) as outsb_pool,
        ):
            bd4 = consts.tile([128, 4], BF16, tag="bd4")
            nc.sync.dma_start(bd4[:], bd4_dram.ap())

            # ---- 8-pair load batches, prefetched 2 groups ahead.
            # Loads sit at the front of each queue's FIFO so the late
            # (scratch/skew) ops behind them can never starve stage 1. ----
            x2tiles = {}
            a5tiles = {}

            def load8(g8):
                # loads pairs for groups g8 (sync queue) and g8+1 (ACT queue)
                p0 = g8 * 4
                x2c8 = x2p.tile([64, 1024], BF16, tag="x2c8")
                nc.gpsimd.dma_start(
                    x2c8[:],
                    bass.AP(d2, p0 * NT, [[128, 64], [NT, 8], [1, 128]]),
                )
                x2tiles[g8] = x2c8
                for half, eng in ((0, nc.sync), (1, nc.scalar)):
                    a54 = apool.tile([64, 4 * A5LEN], BF16, tag="a54")
                    eng.dma_start(
                        a54[:],
                        bass.AP(
                            d1p,
                            (p0 + 4 * half) * X1LEN + XPAD - A5OFF,
                            [[128, 64], [X1LEN, 4], [1, A5LEN]],
                        ),
                    )
                    a5tiles[g8 + half] = a54

            load8(0)
            load8(2)
            load8(4)
            out16 = None
            sks = {}
            for it in range(GROUPS + S7_DELAY):
                if it < GROUPS:
                    if it % 2 == 0 and it + 6 < GROUPS:
                        load8(it + 6)
                    grp = it
                    p0 = grp * 4
                    x2c4 = x2tiles[grp - grp % 2][:, 512 * (grp % 2):
                                                  512 * (grp % 2) + 512]
                    a54 = a5tiles.pop(grp)

                    # ---- stage 1: 16 accumulating matmuls per half ----
                    # both halves in one 2-bank PSUM tile:
                    #   h -> cols 512h + [0, W_H)
                    rps2 = rps_pool.tile([128, 1024], F32, tag="rps2")
                    for h in range(2):
                        for b in range(4):
                            for s in range(4):
                                ja = A5LEN * s + 32 * b + 258 * h
                                nc.tensor.matmul(
                                    rps2[32 * s:32 * s + 32,
                                         512 * h:512 * h + W_H],
                                    x2c4[:, 128 * s + 32 * b:
                                         128 * s + 32 * b + 32],
                                    a54[:, ja:ja + W_H],
                                    start=(b == 0),
                                    stop=(b == 3),
                                    tile_position=(0, 32 * s),
                                )
                    # ---- stage 2: single combined drain (DVE) ----
                    r4sb = r4sb_pool.tile([128, 2 * W_H], BF16, tag="r4sb")
                    nc.vector.tensor_copy(
                        r4sb[:].rearrange("p (h v) -> p h v", h=2),
                        rps2[:].rearrange("p (h v) -> p h v", h=2)
                        [:, :, 0:W_H],
                    )

                    # ---- stage 3: DRAM round trip, skewed re-reads ----
                    scr = scr_pool.tile([128, SCR_W], BF16, tag="scr")
                    nc.gpsimd.dma_start(scr[:, 0:2 * W_H], r4sb[:])
                    sk = sk_pool.tile([128, 2 * SK_W], BF16, tag="sk")
                    base = scr[:]
                    nc.sync.dma_start(
                        sk[:, 0:SK_W],
                        bass.AP(
                            base.tensor, base.offset,
                            [[32 * SCR_W, 4], [SCR_W + 1, 32], [1, SK_W]],
                        ),
                    )
                    nc.scalar.dma_start(
                        sk[:, SK_W:2 * SK_W],
                        bass.AP(
                            base.tensor, base.offset + W_H,
                            [[32 * SCR_W, 4], [SCR_W + 1, 32], [1, SK_W]],
                        ),
                    )
                    sks[grp] = sk

                if it >= S7_DELAY:
                    # ---- stage 4 (delayed): phase-sum, col-tiled over 4
                    # groups.  By now the skew read is long done, so these
                    # never head-of-line-block the tensor queue. ----
                    dgrp = it - S7_DELAY
                    G = dgrp % 4
                    sk = sks.pop(dgrp)
                    if G == 0:
                        out16 = out16_pool.tile([128, 1024], F32, tag="out16")
                    nc.tensor.matmul(
                        out16[32 * G:32 * G + 4, 0:L_H[0]],
                        bd4[:],
                        sk[:, 0:L_H[0]],
                        start=True, stop=True,
                        tile_position=(0, 32 * G),
                    )
                    nc.tensor.matmul(
                        out16[32 * G:32 * G + 4, 512:512 + L_H[1]],
                        bd4[:],
                        sk[:, SK_W:SK_W + L_H[1]],
                        start=True, stop=True,
                        tile_position=(0, 32 * G),
                    )

                    # ---- stage 5: drain + store 16 pairs ----
                    if G == 3:
                        outsb = outsb_pool.tile([128, 516], F32, tag="outsb")
                        nc.vector.tensor_copy(
                            outsb[:].rearrange("p (h v) -> p h v", h=2),
                            out16[:].rearrange("p (h v) -> p h v", h=2)
                            [:, :, 0:L_H[0]],
                        )
                        for Gs in range(4):
                            q0 = (dgrp - 3 + Gs) * 4
                            nc.sync.dma_start(
                                out.ap()[q0:q0 + 4, :],
                                outsb[32 * Gs:32 * Gs + 4, 0:OUT_LEN],
                            )

    return nc


_NC_CACHE = {}


def _get_nc():
    if "nc" not in _NC_CACHE:
        nc = bacc.Bacc("TRN2", target_bir_lowering=False, debug=False)
        _build(nc)
        nc.compile()
        _NC_CACHE["nc"] = nc
    return _NC_CACHE["nc"]


def _make_in_maps(data1: np.ndarray, data2: np.ndarray):
    data1 = np.asarray(data1, dtype=np.float32).astype(ml_dtypes.bfloat16)
    data2 = np.asarray(data2, dtype=np.float32).astype(ml_dtypes.bfloat16)
    in_maps = []
    for k in range(N_CORES):
        d1 = data1[k * NB_PER_CORE:(k + 1) * NB_PER_CORE].reshape(PAIRS, NT)
        d2 = data2[k * NB_PER_CORE:(k + 1) * NB_PER_CORE].reshape(PAIRS, NT)
        d1p = np.zeros((PAIRS, X1LEN), ml_dtypes.bfloat16)
        d1p[:, XPAD:XPAD + NT] = d1
        in_maps.append({"d1p": d1p, "d2": np.ascontiguousarray(d2)})
    return in_maps


def run(data1: np.ndarray, data2: np.ndarray, trace: bool = False):
    nc = _get_nc()
    in_maps = _make_in_maps(data1, data2)
    res = run_bass_kernel_spmd(
        nc, in_maps, core_ids=list(range(N_CORES)), trace=trace
    )
    outs = [res.results[k]["out"].reshape(NB_PER_CORE, NCH, OUT_LEN)
            for k in range(N_CORES)]
    full = np.concatenate(outs, axis=0).astype(np.float32)
    return full, res


def kernel(data1: np.ndarray, data2: np.ndarray) -> np.ndarray:
    full, _ = run(data1, data2, trace=False)
    return full
